# revision 5
# baseline (speedup 1.0000x reference)
"""Luong attention TRN2 Bass kernel.

Problem: B=32, T=2048, H_enc=H_dec=1024.
  proj_keys = einsum('bte,he->bth', keys, W)
  scores    = einsum('bqh,bth->bqt', query, proj_keys) * scale
  alphas    = softmax(where(mask, scores, -inf))
  context   = einsum('bqt,bte->bqe', alphas, values)

Key rewrite: scores[b,t] = (query[b] @ W) . keys[b,t]  (associativity), so the
huge keys projection collapses into a [1,H]x[H,E] query projection per batch.
The kernel is then DMA-bound: stream keys+values once each.

Sharding: data-parallel over batch. 8 cores x 4 batches each. W/scale
replicated. No collectives.
"""

import numpy as np

B, T, E, H = 32, 2048, 1024, 1024
N_CORES = 8
BPC = B // N_CORES          # batches per core
PCH = 128                   # partition chunk
TCH = T // PCH              # 16 t-chunks per batch
KC = H // PCH               # 8 h-chunks
TPD = 2                     # t-chunks per DMA (1MB loads)
NEG_BIG = -1.0e30

_CACHE = {}


def _build_program():
    import concourse.bass as bass
    import concourse.bacc as bacc
    import concourse.mybir as mybir
    import concourse.tile as tile
    from concourse import masks

    f32 = mybir.dt.float32
    u8 = mybir.dt.uint8
    Alu = mybir.AluOpType
    Act = mybir.ActivationFunctionType

    nc = bacc.Bacc(
        "TRN2",
        target_bir_lowering=False,
        debug=False,
        enable_asserts=False,
        num_devices=N_CORES,
    )

    q_d = nc.dram_tensor("query", [BPC, 1, H], f32, kind="ExternalInput")
    k_d = nc.dram_tensor("keys", [BPC, T, E], f32, kind="ExternalInput")
    v_d = nc.dram_tensor("values", [BPC, T, E], f32, kind="ExternalInput")
    m_d = nc.dram_tensor("mask", [BPC, T], u8, kind="ExternalInput")
    w_d = nc.dram_tensor("W", [H, E], f32, kind="ExternalInput")
    s_d = nc.dram_tensor("scale", [1], f32, kind="ExternalInput")
    ctx_d = nc.dram_tensor("context", [BPC, 1, E], f32, kind="ExternalOutput")
    al_d = nc.dram_tensor("alphas", [BPC, 1, T], f32, kind="ExternalOutput")

    with tile.TileContext(nc) as tc:
        with (
            tc.tile_pool(name="consts", bufs=1) as consts,
            tc.tile_pool(name="qwp", bufs=1) as qwp,
            tc.tile_pool(name="keysp", bufs=3) as keysp,
            tc.tile_pool(name="valsp", bufs=4) as valsp,
            tc.tile_pool(name="smallp", bufs=1) as smallp,
        ):
            # ---------------- phase 0: qW = (query @ W) * scale ----------
            identity = consts.tile([PCH, PCH], f32)
            masks.make_identity(nc, identity[:])

            scale4 = consts.tile([BPC, 1], f32)
            for b in range(BPC):
                nc.sync.dma_start(scale4[b : b + 1, :], s_d[0:1])

            q_nat = consts.tile([BPC, H], f32)
            nc.sync.dma_start(q_nat[:], q_d[:, 0, :])

            # selection matrices for the broadcast matmul: sel_b[k,m] = (k==b)
            ones_row = consts.tile([1, PCH], f32)
            nc.vector.memset(ones_row[:], 1.0)
            sels = []
            for b in range(BPC):
                sel = consts.tile([BPC, PCH], f32, name=f"sel{b}")
                nc.vector.memset(sel[:], 0.0)
                nc.sync.dma_start(sel[b : b + 1, :], ones_row[:])
                sels.append(sel)

            qT_sb = []
            with tc.tile_pool(name="ps_qt", bufs=2, space="PSUM") as ps_qt:
                for k in range(KC):
                    qT_ps = ps_qt.tile([PCH, BPC], f32, tag="qt")
                    nc.tensor.transpose(
                        qT_ps[:], q_nat[:, k * PCH : (k + 1) * PCH], identity[:BPC, :BPC]
                    )
                    qT_k = consts.tile([PCH, BPC], f32, name=f"qT{k}")
                    nc.scalar.copy(qT_k[:], qT_ps[:])
                    qT_sb.append(qT_k)

            with tc.tile_pool(name="wp", bufs=1) as wp:
                w_tiles = []
                for k in range(KC):
                    w_k = wp.tile([PCH, E], f32, name=f"w{k}")
                    nc.sync.dma_start(w_k[:], w_d[k * PCH : (k + 1) * PCH, :])
                    w_tiles.append(w_k)

                qw_sb = qwp.tile([BPC, E], f32)
                with tc.tile_pool(name="ps_qw", bufs=1, space="PSUM") as ps_qw:
                    qw_ps = ps_qw.tile([BPC, E], f32)
                    for nh in range(2):
                        for k in range(KC):
                            nc.tensor.matmul(
                                qw_ps[:, nh * 512 : (nh + 1) * 512],
                                qT_sb[k][:],
                                w_tiles[k][:, nh * 512 : (nh + 1) * 512],
                                start=(k == 0),
                                stop=(k == KC - 1),
                            )
                    # apply attention scale while evacuating PSUM
                    nc.scalar.activation(
                        qw_sb[:], qw_ps[:], Act.Copy, bias=0.0, scale=scale4[:]
                    )

            # broadcast each batch's qW row across 128 partitions via PE
            qw_bc = []
            with tc.tile_pool(name="ps_bc", bufs=2, space="PSUM") as ps_bc:
                for b in range(BPC):
                    qb = qwp.tile([PCH, E], f32, name=f"qwbc{b}")
                    for nh in range(2):
                        bc_ps = ps_bc.tile([PCH, 512], f32, tag="bc")
                        nc.tensor.matmul(
                            bc_ps[:],
                            sels[b][:],
                            qw_sb[:, nh * 512 : (nh + 1) * 512],
                            start=True,
                            stop=True,
                        )
                        nc.scalar.copy(qb[:, nh * 512 : (nh + 1) * 512], bc_ps[:])
                    qw_bc.append(qb)

            # mask load + convert
            mask_u8 = smallp.tile([BPC, T], u8)
            nc.sync.dma_start(mask_u8[:], m_d[:])
            mask_f = smallp.tile([BPC, T], f32)
            nc.vector.tensor_copy(mask_f[:], mask_u8[:])

            # ---------------- phase 1: scores -----------------------------
            dummy = smallp.tile([PCH, 1], f32)
            scores_b = [
                smallp.tile([PCH, TCH], f32, name=f"scores{b}") for b in range(BPC)
            ]
            for b in range(BPC):
                for dj in range(TCH // TPD):
                    t0 = dj * TPD * PCH
                    keys_tile = keysp.tile([PCH, TPD, E], f32, tag="keys")
                    nc.sync.dma_start(
                        keys_tile[:],
                        k_d[b : b + 1, t0 : t0 + TPD * PCH, :].rearrange(
                            "o (c p) e -> (o p) c e", p=PCH
                        ),
                    )
                    for c in range(TPD):
                        i = dj * TPD + c
                        nc.vector.scalar_tensor_tensor(
                            out=dummy.broadcast_to((PCH, E)),
                            in0=keys_tile[:, c, :],
                            scalar=1.0,
                            in1=qw_bc[b][:],
                            op0=Alu.mult,
                            op1=Alu.mult,
                            accum_out=scores_b[b][:, i : i + 1],
                        )

            # ---------------- phase 2: assemble scores rows ---------------
            scoresR = smallp.tile([BPC, T], f32)
            with tc.tile_pool(name="ps_st", bufs=2, space="PSUM") as ps_st:
                for b in range(BPC):
                    st_ps = ps_st.tile([TCH, PCH], f32, tag="st")
                    nc.tensor.transpose(st_ps[:], scores_b[b][:], identity[:])
                    st_sb = smallp.tile([TCH, PCH], f32, name=f"st{b}")
                    nc.scalar.copy(st_sb[:], st_ps[:])
                    nc.sync.dma_start(scoresR[b : b + 1, :], st_sb[:])

            # ---------------- phase 3: masked softmax ---------------------
            pen = smallp.tile([BPC, T], f32)
            nc.vector.tensor_scalar(
                out=pen[:],
                in0=mask_f[:],
                scalar1=1.0,
                scalar2=-NEG_BIG,
                op0=Alu.subtract,
                op1=Alu.mult,
            )
            sm = smallp.tile([BPC, T], f32)
            nc.vector.tensor_add(sm[:], scoresR[:], pen[:])
            mx = smallp.tile([BPC, 1], f32)
            nc.vector.tensor_reduce(mx[:], sm[:], axis=mybir.AxisListType.X, op=Alu.max)
            negmx = smallp.tile([BPC, 1], f32)
            nc.vector.tensor_scalar_mul(negmx[:], mx[:], -1.0)
            exps = smallp.tile([BPC, T], f32)
            sumex = smallp.tile([BPC, 1], f32)
            nc.scalar.activation(
                exps[:], sm[:], Act.Exp, bias=negmx[:], scale=1.0, accum_out=sumex[:]
            )
            rs = smallp.tile([BPC, 1], f32)
            nc.vector.reciprocal(rs[:], sumex[:])
            alphas_row = smallp.tile([BPC, T], f32)
            nc.vector.tensor_scalar_mul(alphas_row[:], exps[:], rs[:])
            nc.sync.dma_start(al_d[:, 0, :], alphas_row[:])

            # ---------------- phase 4: alphasT ----------------------------
            aT = []
            with tc.tile_pool(name="ps_at", bufs=2, space="PSUM") as ps_at:
                for i in range(TCH):
                    aT_ps = ps_at.tile([PCH, BPC], f32, tag="at")
                    nc.tensor.transpose(
                        aT_ps[:],
                        alphas_row[:, i * PCH : (i + 1) * PCH],
                        identity[:BPC, :BPC],
                    )
                    aT_i = smallp.tile([PCH, BPC], f32, name=f"aT{i}")
                    nc.scalar.copy(aT_i[:], aT_ps[:])
                    aT.append(aT_i)

            # ---------------- phase 5: context = alphas @ values ----------
            with tc.tile_pool(name="ps_ctx", bufs=2, space="PSUM") as ps_ctx:
                for b in range(BPC):
                    ctx_ps = ps_ctx.tile([1, E], f32, tag="ctx")
                    for dj in range(TCH // TPD):
                        t0 = dj * TPD * PCH
                        vals_tile = valsp.tile([PCH, TPD, E], f32, tag="vals")
                        nc.sync.dma_start(
                            vals_tile[:],
                            v_d[b : b + 1, t0 : t0 + TPD * PCH, :].rearrange(
                                "o (c p) e -> (o p) c e", p=PCH
                            ),
                        )
                        for c in range(TPD):
                            i = dj * TPD + c
                            for nh in range(2):
                                nc.tensor.matmul(
                                    ctx_ps[:, nh * 512 : (nh + 1) * 512],
                                    aT[i][:, b : b + 1],
                                    vals_tile[:, c, nh * 512 : (nh + 1) * 512],
                                    start=(i == 0),
                                    stop=(i == TCH - 1),
                                )
                    ctx_sb = smallp.tile([1, E], f32, name=f"ctx{b}")
                    nc.scalar.copy(ctx_sb[:], ctx_ps[:])
                    nc.sync.dma_start(ctx_d[b : b + 1, 0, :], ctx_sb[:])

    nc.compile()
    return nc


def get_program():
    if "nc" not in _CACHE:
        _CACHE["nc"] = _build_program()
    return _CACHE["nc"]


def kernel(query, mask, values, keys, W, scale):
    from concourse import bass_utils

    nc = get_program()

    mask_u8 = np.ascontiguousarray(mask).view(np.uint8)
    in_maps = []
    for c in range(N_CORES):
        sl = slice(c * BPC, (c + 1) * BPC)
        in_maps.append(
            {
                "query": np.ascontiguousarray(query[sl], dtype=np.float32),
                "keys": np.ascontiguousarray(keys[sl], dtype=np.float32),
                "values": np.ascontiguousarray(values[sl], dtype=np.float32),
                "mask": np.ascontiguousarray(mask_u8[sl]),
                "W": np.ascontiguousarray(W, dtype=np.float32),
                "scale": np.ascontiguousarray(scale, dtype=np.float32),
            }
        )

    res = bass_utils.run_bass_kernel_spmd(nc, in_maps, core_ids=list(range(N_CORES)))
    context = np.concatenate([r["context"] for r in res.results], axis=0)
    alphas = np.concatenate([r["alphas"] for r in res.results], axis=0)
    return context, alphas


# revision 8
# speedup vs baseline: 1.0120x; 1.0120x over previous
"""Luong attention TRN2 Bass kernel.

Problem: B=32, T=2048, H_enc=H_dec=1024.
  proj_keys = einsum('bte,he->bth', keys, W)
  scores    = einsum('bqh,bth->bqt', query, proj_keys) * scale
  alphas    = softmax(where(mask, scores, -inf))
  context   = einsum('bqt,bte->bqe', alphas, values)

Key rewrite: scores[b,t] = (query[b] @ W) . keys[b,t]  (associativity), so the
huge keys projection collapses into a [1,H]x[H,E] query projection per batch.
The kernel is then DMA-bound: stream keys+values once each.

Sharding: data-parallel over batch. 8 cores x 4 batches each. W/scale
replicated. No collectives.
"""

import numpy as np

B, T, E, H = 32, 2048, 1024, 1024
N_CORES = 8
BPC = B // N_CORES          # batches per core
PCH = 128                   # partition chunk
TCH = T // PCH              # 16 t-chunks per batch
KC = H // PCH               # 8 h-chunks
TPD = 4                     # t-chunks per DMA (2MB f32 loads)
NEG_BIG = -1.0e30

_CACHE = {}


def _build_program():
    import concourse.bass as bass
    import concourse.bacc as bacc
    import concourse.mybir as mybir
    import concourse.tile as tile
    from concourse import masks

    f32 = mybir.dt.float32
    bf16 = mybir.dt.bfloat16
    u8 = mybir.dt.uint8
    Alu = mybir.AluOpType
    Act = mybir.ActivationFunctionType

    nc = bacc.Bacc(
        "TRN2",
        target_bir_lowering=False,
        debug=False,
        enable_asserts=False,
        num_devices=N_CORES,
    )

    q_d = nc.dram_tensor("query", [BPC, 1, H], f32, kind="ExternalInput")
    k_d = nc.dram_tensor("keys", [BPC, T, E], f32, kind="ExternalInput")
    v_d = nc.dram_tensor("values", [BPC, T, E], f32, kind="ExternalInput")
    m_d = nc.dram_tensor("mask", [BPC, T], u8, kind="ExternalInput")
    w_d = nc.dram_tensor("W", [H, E], f32, kind="ExternalInput")
    s_d = nc.dram_tensor("scale", [1], f32, kind="ExternalInput")
    ctx_d = nc.dram_tensor("context", [BPC, 1, E], f32, kind="ExternalOutput")
    al_d = nc.dram_tensor("alphas", [BPC, 1, T], f32, kind="ExternalOutput")

    with tile.TileContext(nc) as tc:
        with (
            tc.tile_pool(name="consts", bufs=1) as consts,
            tc.tile_pool(name="qwp", bufs=1) as qwp,
            tc.tile_pool(name="keysp", bufs=2) as keysp,
            tc.tile_pool(name="valsp", bufs=6) as valsp,
            tc.tile_pool(name="smallp", bufs=1) as smallp,
        ):
            # ---------------- phase 0: qW = (query @ W) * scale ----------
            identity = consts.tile([PCH, PCH], f32)
            masks.make_identity(nc, identity[:])
            identity_bf = consts.tile([PCH, PCH], bf16)
            masks.make_identity(nc, identity_bf[:])

            scale4 = consts.tile([BPC, 1], f32)
            for b in range(BPC):
                nc.scalar.dma_start(scale4[b : b + 1, :], s_d[0:1])

            q_nat = consts.tile([BPC, H], f32)
            nc.scalar.dma_start(q_nat[:], q_d[:, 0, :])

            # selection matrices for the broadcast matmul: sel_b[k,m] = (k==b)
            ones_row = consts.tile([1, PCH], f32)
            nc.vector.memset(ones_row[:], 1.0)
            sels = []
            for b in range(BPC):
                sel = consts.tile([BPC, PCH], f32, name=f"sel{b}")
                nc.vector.memset(sel[:], 0.0)
                nc.scalar.dma_start(sel[b : b + 1, :], ones_row[:])
                sels.append(sel)

            qT_sb = []
            with tc.tile_pool(name="ps_qt", bufs=2, space="PSUM") as ps_qt:
                for k in range(KC):
                    qT_ps = ps_qt.tile([PCH, BPC], f32, tag="qt")
                    nc.tensor.transpose(
                        qT_ps[:], q_nat[:, k * PCH : (k + 1) * PCH], identity[:BPC, :BPC]
                    )
                    qT_k = consts.tile([PCH, BPC], f32, name=f"qT{k}")
                    nc.scalar.copy(qT_k[:], qT_ps[:])
                    qT_sb.append(qT_k)

            with tc.tile_pool(name="wp", bufs=3) as wp:
                qw_sb = qwp.tile([BPC, E], f32)
                with tc.tile_pool(name="ps_qw", bufs=1, space="PSUM") as ps_qw:
                    qw_ps = ps_qw.tile([BPC, E], f32)
                    for k in range(KC):
                        w_k = wp.tile([PCH, E], f32, tag="w")
                        nc.scalar.dma_start(w_k[:], w_d[k * PCH : (k + 1) * PCH, :])
                        for nh in range(2):
                            nc.tensor.matmul(
                                qw_ps[:, nh * 512 : (nh + 1) * 512],
                                qT_sb[k][:],
                                w_k[:, nh * 512 : (nh + 1) * 512],
                                start=(k == 0),
                                stop=(k == KC - 1),
                            )
                    # apply attention scale while evacuating PSUM
                    nc.scalar.activation(
                        qw_sb[:], qw_ps[:], Act.Copy, bias=0.0, scale=scale4[:]
                    )

            # broadcast each batch's qW row across 128 partitions via PE
            qw_bc = []
            with tc.tile_pool(name="ps_bc", bufs=2, space="PSUM") as ps_bc:
                for b in range(BPC):
                    qb = qwp.tile([PCH, E], f32, name=f"qwbc{b}")
                    for nh in range(2):
                        bc_ps = ps_bc.tile([PCH, 512], f32, tag="bc")
                        nc.tensor.matmul(
                            bc_ps[:],
                            sels[b][:],
                            qw_sb[:, nh * 512 : (nh + 1) * 512],
                            start=True,
                            stop=True,
                        )
                        nc.scalar.copy(qb[:, nh * 512 : (nh + 1) * 512], bc_ps[:])
                    qw_bc.append(qb)

            # mask load + convert
            mask_u8 = smallp.tile([BPC, T], u8)
            nc.scalar.dma_start(mask_u8[:], m_d[:])
            mask_f = smallp.tile([BPC, T], f32)
            nc.vector.tensor_copy(mask_f[:], mask_u8[:])

            # ---------------- phase 1: scores -----------------------------
            dummy = smallp.tile([PCH, 1], f32)
            scores_b = [
                smallp.tile([PCH, TCH], f32, name=f"scores{b}") for b in range(BPC)
            ]
            for b in range(BPC):
                for dj in range(TCH // TPD):
                    t0 = dj * TPD * PCH
                    keys_tile = keysp.tile([PCH, TPD, E], f32, tag="keys")
                    nc.sync.dma_start(
                        keys_tile[:],
                        k_d[b : b + 1, t0 : t0 + TPD * PCH, :].rearrange(
                            "o (c p) e -> (o p) c e", p=PCH
                        ),
                    )
                    for c in range(TPD):
                        i = dj * TPD + c
                        nc.vector.scalar_tensor_tensor(
                            out=dummy.broadcast_to((PCH, E)),
                            in0=keys_tile[:, c, :],
                            scalar=1.0,
                            in1=qw_bc[b][:],
                            op0=Alu.mult,
                            op1=Alu.mult,
                            accum_out=scores_b[b][:, i : i + 1],
                        )

            # ---------------- phase 2: assemble scores rows ---------------
            scoresR = smallp.tile([BPC, T], f32)
            with tc.tile_pool(name="ps_st", bufs=2, space="PSUM") as ps_st:
                for b in range(BPC):
                    st_ps = ps_st.tile([TCH, PCH], f32, tag="st")
                    nc.tensor.transpose(st_ps[:], scores_b[b][:], identity[:])
                    st_sb = smallp.tile([TCH, PCH], f32, name=f"st{b}")
                    nc.scalar.copy(st_sb[:], st_ps[:])
                    nc.sync.dma_start(scoresR[b : b + 1, :], st_sb[:])

            # ---------------- phase 3: masked softmax ---------------------
            # pen = (mask-1)*1e30 in place of mask_f
            nc.vector.tensor_scalar(
                out=mask_f[:],
                in0=mask_f[:],
                scalar1=1.0,
                scalar2=-NEG_BIG,
                op0=Alu.subtract,
                op1=Alu.mult,
            )
            sm = smallp.tile([BPC, T], f32)
            nc.vector.tensor_add(sm[:], scoresR[:], mask_f[:])
            mx = smallp.tile([BPC, 1], f32)
            nc.vector.tensor_reduce(mx[:], sm[:], axis=mybir.AxisListType.X, op=Alu.max)
            negmx = smallp.tile([BPC, 1], f32)
            nc.vector.tensor_scalar_mul(negmx[:], mx[:], -1.0)
            sumex = smallp.tile([BPC, 1], f32)
            nc.scalar.activation(
                sm[:], sm[:], Act.Exp, bias=negmx[:], scale=1.0, accum_out=sumex[:]
            )
            rs = smallp.tile([BPC, 1], f32)
            nc.vector.reciprocal(rs[:], sumex[:])
            alphas_row = sm
            nc.vector.tensor_scalar_mul(alphas_row[:], sm[:], rs[:])
            nc.sync.dma_start(al_d[:, 0, :], alphas_row[:])
            alphas_bf = smallp.tile([BPC, T], bf16)
            nc.vector.tensor_copy(alphas_bf[:], alphas_row[:])

            # ---------------- phase 4: alphasT ----------------------------
            aT = []
            with tc.tile_pool(name="ps_at", bufs=2, space="PSUM") as ps_at:
                for i in range(TCH):
                    aT_ps = ps_at.tile([PCH, BPC], bf16, tag="at")
                    nc.tensor.transpose(
                        aT_ps[:],
                        alphas_bf[:, i * PCH : (i + 1) * PCH],
                        identity_bf[:BPC, :BPC],
                    )
                    aT_i = smallp.tile([PCH, BPC], bf16, name=f"aT{i}")
                    nc.scalar.copy(aT_i[:], aT_ps[:])
                    aT.append(aT_i)

            # ---------------- phase 5: context = alphas @ values ----------
            with tc.tile_pool(name="ps_ctx", bufs=2, space="PSUM") as ps_ctx:
                for b in range(BPC):
                    ctx_ps = ps_ctx.tile([1, E], f32, tag="ctx")
                    for dj in range(TCH // TPD):
                        t0 = dj * TPD * PCH
                        vals_tile = valsp.tile([PCH, TPD, E], bf16, tag="vals")
                        nc.gpsimd.dma_start(
                            vals_tile[:],
                            v_d[b : b + 1, t0 : t0 + TPD * PCH, :].rearrange(
                                "o (c p) e -> (o p) c e", p=PCH
                            ),
                        )
                        for c in range(TPD):
                            i = dj * TPD + c
                            for nh in range(2):
                                nc.tensor.matmul(
                                    ctx_ps[:, nh * 512 : (nh + 1) * 512],
                                    aT[i][:, b : b + 1],
                                    vals_tile[:, c, nh * 512 : (nh + 1) * 512],
                                    start=(i == 0),
                                    stop=(i == TCH - 1),
                                )
                    ctx_sb = smallp.tile([1, E], f32, name=f"ctx{b}")
                    nc.scalar.copy(ctx_sb[:], ctx_ps[:])
                    nc.sync.dma_start(ctx_d[b : b + 1, 0, :], ctx_sb[:])

    nc.compile()
    return nc


def get_program():
    if "nc" not in _CACHE:
        _CACHE["nc"] = _build_program()
    return _CACHE["nc"]


def kernel(query, mask, values, keys, W, scale):
    from concourse import bass_utils

    nc = get_program()

    mask_u8 = np.ascontiguousarray(mask).view(np.uint8)
    in_maps = []
    for c in range(N_CORES):
        sl = slice(c * BPC, (c + 1) * BPC)
        in_maps.append(
            {
                "query": np.ascontiguousarray(query[sl], dtype=np.float32),
                "keys": np.ascontiguousarray(keys[sl], dtype=np.float32),
                "values": np.ascontiguousarray(values[sl], dtype=np.float32),
                "mask": np.ascontiguousarray(mask_u8[sl]),
                "W": np.ascontiguousarray(W, dtype=np.float32),
                "scale": np.ascontiguousarray(scale, dtype=np.float32),
            }
        )

    res = bass_utils.run_bass_kernel_spmd(nc, in_maps, core_ids=list(range(N_CORES)))
    context = np.concatenate([r["context"] for r in res.results], axis=0)
    alphas = np.concatenate([r["alphas"] for r in res.results], axis=0)
    return context, alphas
